# revision 4
# baseline (speedup 1.0000x reference)
"""Trainium2 Bass kernel for nn_Attention2 (sparse_attention).

Self-contained: batch-parallel over 8 NeuronCores (B=8 -> 1 batch/core).
Per core:
  att_input^T = (Q @ W)^T   via fp16 3-term split matmuls (fp32-quality)
  scores      = att_input @ K^T + mask_bias   (fp16 3-term split + bias-MM)
  softmax / sigmoid(=0.5+0.5*tanh(x/2)) / hard=(s>0) on DVE+ACT
  att_out_sm  = probs @ V,  att_out_sg = hard @ V   (fp16 matmuls)
Q^T, K^T obtained with bit-exact PE transposes of the fp16 hi/lo splits.
"""
import numpy as np

import bass_rust
import concourse.bass as bass
import concourse.mybir as mybir
import concourse.tile as tile
from concourse.bass_utils import run_bass_kernel_spmd
from concourse.masks import make_identity

F32 = mybir.dt.float32
F16 = mybir.dt.float16
I32 = mybir.dt.int32
AX = mybir.AxisListType.X
OP = mybir.AluOpType
ACTF = mybir.ActivationFunctionType

LQ = LK = 2048
H = 1024
NB = 8          # cores / batch
QS = 512        # q super-tile
NEG = -60000.0  # mask bias (fp16-representable; exp/tanh saturate exactly)


def _split_excess_waits(nc, cap_default=1, cap_event=2):
    """walrus here encodes <=1 sync-wait per instruction (2 for
    EventSemaphore); spill extras onto standalone carriers."""
    n = [0]
    for f in nc.m.functions:
        for bb in f.blocks:
            insts = bb.instructions
            i = 0
            while i < len(insts):
                inst = insts[i]
                si = inst.sync_info
                if si is None:
                    i += 1
                    continue
                waits = list(si.on_wait)
                cap = (
                    cap_event
                    if isinstance(inst, mybir.InstEventSemaphore)
                    else cap_default
                )
                if len(waits) <= cap:
                    i += 1
                    continue
                sem = [w for w in waits if w.sync_type == "semaphore"]
                pin = [w for w in waits if w.sync_type != "semaphore"]
                nk = max(cap - len(pin), 0)
                keep = pin + sem[len(sem) - nk :]
                spill = sem[: len(sem) - nk]
                if not spill:
                    i += 1
                    continue
                carriers = []
                for j in range(0, len(spill), cap_event):
                    es = mybir.InstEventSemaphore(name=f"I-wspill-{n[0]}")
                    n[0] += 1
                    es.engine = inst.engine
                    es.sync_info = bass_rust.SyncInfo(
                        on_wait=spill[j : j + cap_event], on_update=[]
                    )
                    nc.register_instruction(es)
                    carriers.append(es)
                inst.sync_info = bass_rust.SyncInfo(
                    on_wait=keep, on_update=list(si.on_update)
                )
                insts[i:i] = carriers
                i += 1 + len(carriers)


def build_nc():
    nc = bass.Bass("TRN2", target_bir_lowering=False)
    q_d = nc.dram_tensor("q", [LQ, H], F32, kind="ExternalInput")
    k_d = nc.dram_tensor("k", [LK, H], F32, kind="ExternalInput")
    v_d = nc.dram_tensor("v", [LK, H], F32, kind="ExternalInput")
    w_d = nc.dram_tensor("w", [H, H], F32, kind="ExternalInput")
    m_d = nc.dram_tensor("mask", [1, LK], I32, kind="ExternalInput")
    o_sm = nc.dram_tensor("o_sm", [LQ, H], F32, kind="ExternalOutput")
    o_sg = nc.dram_tensor("o_sg", [LQ, H], F32, kind="ExternalOutput")
    o_psm = nc.dram_tensor("o_psm", [LQ, LK], F32, kind="ExternalOutput")
    o_psg = nc.dram_tensor("o_psg", [LQ, LK], F32, kind="ExternalOutput")
    # DRAM staging for K^T splits: [dpart, dchunk, k]
    kt_h_d = nc.dram_tensor("kt_h", [128, 8, LK], F16)
    kt_l_d = nc.dram_tensor("kt_l", [128, 8, LK], F16)

    with tile.TileContext(nc) as tc:
        with (
            tc.tile_pool(name="const", bufs=1) as cpool,
            tc.tile_pool(name="wres", bufs=1) as wpool,
            tc.tile_pool(name="vres", bufs=1) as vpool,
            tc.tile_pool(name="stage", bufs=2) as spool,
            tc.tile_pool(name="hl", bufs=2) as hlpool,
            tc.tile_pool(name="qtp", bufs=1) as qtpool,
            tc.tile_pool(name="ktp", bufs=1) as ktpool,
            tc.tile_pool(name="work", bufs=1) as wkpool,
            tc.tile_pool(name="outp", bufs=1) as opool,
            tc.tile_pool(name="ps_mm", bufs=2, space="PSUM") as ps_mm,
            tc.tile_pool(name="ps_att", bufs=1, space="PSUM") as ps_att,
            tc.tile_pool(name="ps_tr", bufs=2, space="PSUM") as ps_tr,
        ):
            ident = cpool.tile([128, 128], F16)
            make_identity(nc, ident[:])
            ones1 = cpool.tile([1, 128], F16)
            nc.vector.memset(ones1[:], 1.0)
            # mask -> fp16 bias row: (mask < 0.5) * NEG
            mrow = spool.tile([1, LK], I32, name="mrow", bufs=1)
            nc.sync.dma_start(mrow[:], m_d.ap()[:])
            mrow32 = spool.tile([1, LK], F32, name="mrow32", bufs=1)
            nc.vector.tensor_copy(mrow32[:], mrow[:])
            brow = cpool.tile([1, LK], F16)
            nc.vector.tensor_scalar(
                out=brow[:], in0=mrow32[:], scalar1=0.5, scalar2=NEG,
                op0=OP.is_lt, op1=OP.mult,
            )

            # --- W splits (resident): 8 x [128h, 1024d] fp16 hi/lo
            w_h, w_l = [], []
            for i in range(8):
                wst = spool.tile([128, H], F32, name="xst")
                nc.sync.dma_start(wst[:], w_d.ap()[i * 128 : (i + 1) * 128, :])
                wh = wpool.tile([128, H], F16, name=f"wh{i}")
                nc.vector.tensor_copy(wh[:], wst[:])
                wl = wpool.tile([128, H], F16, name=f"wl{i}")
                nc.vector.tensor_tensor(
                    out=wl[:], in0=wst[:], in1=wh[:], op=OP.subtract
                )
                w_h.append(wh)
                w_l.append(wl)

            # --- V fp16 (resident): 16 x [128k, 1024h]
            v16 = []
            for i in range(16):
                vst = spool.tile([128, H], F32, name="xst")
                nc.sync.dma_start(vst[:], v_d.ap()[i * 128 : (i + 1) * 128, :])
                vt = vpool.tile([128, H], F16, name=f"v16_{i}")
                nc.vector.tensor_copy(vt[:], vst[:])
                v16.append(vt)

            # --- K split + transpose -> DRAM staging kt_h/kt_l
            for i in range(16):
                kst = spool.tile([128, H], F32, name="xst")
                nc.sync.dma_start(kst[:], k_d.ap()[i * 128 : (i + 1) * 128, :])
                kh = hlpool.tile([128, H], F16, name="hsp")
                nc.vector.tensor_copy(kh[:], kst[:])
                kl = hlpool.tile([128, H], F16, name="lsp")
                nc.vector.tensor_tensor(
                    out=kl[:], in0=kst[:], in1=kh[:], op=OP.subtract
                )
                for src, dst in ((kh, kt_h_d), (kl, kt_l_d)):
                    pt = ps_tr.tile([128, 1024], F16, name="trps")
                    for j in range(8):
                        nc.tensor.transpose(
                            pt[:, j * 128 : (j + 1) * 128],
                            src[:, j * 128 : (j + 1) * 128],
                            ident[:],
                        )
                    se = wkpool.tile([128, 1024], F16, name="ktev", bufs=2)
                    nc.vector.tensor_copy(se[:], pt[:])
                    nc.sync.dma_start(
                        dst.ap()[:, :, i * 128 : (i + 1) * 128],
                        se.rearrange("p (c f) -> p c f", c=8),
                    )

            # --- main loop over q super-tiles
            for s in range(LQ // QS):
                # Q^T splits for this super: [128h, 8hc, 512q] fp16 hi/lo
                qt_h = qtpool.tile([128, 8, QS], F16, name="qt_h")
                qt_l = qtpool.tile([128, 8, QS], F16, name="qt_l")
                for t in range(QS // 128):
                    qrow = s * QS + t * 128
                    qst = spool.tile([128, H], F32, name="xst")
                    nc.sync.dma_start(qst[:], q_d.ap()[qrow : qrow + 128, :])
                    qh = hlpool.tile([128, H], F16, name="hsp")
                    nc.vector.tensor_copy(qh[:], qst[:])
                    ql = hlpool.tile([128, H], F16, name="lsp")
                    nc.vector.tensor_tensor(
                        out=ql[:], in0=qst[:], in1=qh[:], op=OP.subtract
                    )
                    for src, dst in ((qh, qt_h), (ql, qt_l)):
                        pt = ps_tr.tile([128, 1024], F16, name="trps")
                        for j in range(8):
                            nc.tensor.transpose(
                                pt[:, j * 128 : (j + 1) * 128],
                                src[:, j * 128 : (j + 1) * 128],
                                ident[:],
                            )
                        nc.vector.tensor_copy(
                            dst[:, :, t * 128 : (t + 1) * 128],
                            pt.rearrange("p (c f) -> p c f", c=8),
                        )

                # att_input^T splits: [128d, 8dc, 512q] fp16 hi/lo
                a_h = qtpool.tile([128, 8, QS], F16, name="a_h")
                a_l = qtpool.tile([128, 8, QS], F16, name="a_l")
                for j in range(8):
                    pm = ps_mm.tile([128, QS], F32, name="mmps")
                    nmm = 24
                    idx = 0
                    for i in range(8):
                        for lt, rt in (
                            (w_h[i], qt_h), (w_h[i], qt_l), (w_l[i], qt_h)
                        ):
                            nc.tensor.matmul(
                                pm[:],
                                lt[:, j * 128 : (j + 1) * 128],
                                rt[:, i, :],
                                start=(idx == 0),
                                stop=(idx == nmm - 1),
                            )
                            idx += 1
                    nc.vector.tensor_copy(a_h[:, j, :], pm[:])
                    nc.vector.tensor_tensor(
                        out=a_l[:, j, :], in0=pm[:], in1=a_h[:, j, :],
                        op=OP.subtract,
                    )

                # per half-super (2 q-tiles): stream KT, scores, softmax, outs
                for hf in range(2):
                    scores = []
                    for th in range(2):
                        scores.append(
                            wkpool.tile([128, LK], F32, name=f"sc{th}")
                        )
                    for c in range(4):  # k-chunks of 512
                        kth = ktpool.tile([128, 8, 512], F16, name="kth")
                        nc.sync.dma_start(
                            kth[:], kt_h_d.ap()[:, :, c * 512 : (c + 1) * 512]
                        )
                        ktl = ktpool.tile([128, 8, 512], F16, name="ktl")
                        nc.sync.dma_start(
                            ktl[:], kt_l_d.ap()[:, :, c * 512 : (c + 1) * 512]
                        )
                        for th in range(2):
                            t = hf * 2 + th
                            pm = ps_mm.tile([128, 512], F32, name="mmps")
                            nc.tensor.matmul(
                                pm[:], ones1[:],
                                brow[:, c * 512 : (c + 1) * 512],
                                start=True, stop=False,
                            )
                            idx = 0
                            for j in range(8):
                                for lt, rt in (
                                    (a_h, kth), (a_h, ktl), (a_l, kth)
                                ):
                                    nc.tensor.matmul(
                                        pm[:],
                                        lt[:, j, t * 128 : (t + 1) * 128],
                                        rt[:, j, :],
                                        start=False,
                                        stop=(idx == 23),
                                    )
                                    idx += 1
                            nc.scalar.activation(
                                scores[th][:, c * 512 : (c + 1) * 512],
                                pm[:], ACTF.Copy,
                            )
                    for th in range(2):
                        t = hf * 2 + th
                        qrow = s * QS + t * 128
                        sc = scores[th]
                        negmax = wkpool.tile([128, 1], F32, name="negmax")
                        nc.vector.tensor_reduce(
                            out=negmax[:], in_=sc[:], axis=AX,
                            op=OP.max, negate=True,
                        )
                        hard = wkpool.tile([128, LK], F16, name="hard")
                        nc.vector.tensor_scalar(
                            out=hard[:], in0=sc[:], scalar1=0.0,
                            scalar2=None, op0=OP.is_gt,
                        )
                        sig = opool.tile([128, LK], F32, name="sig")
                        nc.scalar.activation(
                            sig[:], sc[:], ACTF.Tanh, scale=0.5
                        )
                        nc.vector.tensor_scalar(
                            out=sig[:], in0=sig[:], scalar1=0.5,
                            scalar2=0.5, op0=OP.mult, op1=OP.add,
                        )
                        nc.sync.dma_start(
                            o_psg.ap()[qrow : qrow + 128, :], sig[:]
                        )
                        sumexp = wkpool.tile([128, 1], F32, name="sumexp")
                        nc.scalar.activation(
                            sc[:], sc[:], ACTF.Exp, bias=negmax[:],
                            accum_out=sumexp[:],
                        )
                        rsum = wkpool.tile([128, 1], F32, name="rsum")
                        nc.vector.reciprocal(rsum[:], sumexp[:])
                        p32 = opool.tile([128, LK], F32, name="p32")
                        nc.vector.tensor_scalar(
                            out=p32[:], in0=sc[:], scalar1=rsum[:],
                            scalar2=None, op0=OP.mult,
                        )
                        nc.sync.dma_start(
                            o_psm.ap()[qrow : qrow + 128, :], p32[:]
                        )
                        p16 = wkpool.tile([128, LK], F16, name="p16")
                        nc.vector.tensor_scalar(
                            out=p16[:], in0=sc[:], scalar1=rsum[:],
                            scalar2=None, op0=OP.mult,
                        )
                        # transpose probs/hard -> [128k, 16kc, 128q]
                        pT = wkpool.tile([128, 16, 128], F16, name="pT")
                        hT = wkpool.tile([128, 16, 128], F16, name="hT")
                        for src, dst in ((p16, pT), (hard, hT)):
                            for g in range(2):
                                pt = ps_tr.tile([128, 1024], F16, name="trps")
                                for j in range(8):
                                    cc = g * 8 + j
                                    nc.tensor.transpose(
                                        pt[:, j * 128 : (j + 1) * 128],
                                        src[:, cc * 128 : (cc + 1) * 128],
                                        ident[:],
                                    )
                                nc.vector.tensor_copy(
                                    dst[:, g * 8 : (g + 1) * 8, :],
                                    pt.rearrange("p (c f) -> p c f", c=8),
                                )
                        # mm3 / mm4
                        for mat, od in ((pT, o_sm), (hT, o_sg)):
                            pa = ps_att.tile([128, H], F32, name="attps")
                            for c in range(16):
                                for half in range(2):
                                    nc.tensor.matmul(
                                        pa[:, half * 512 : (half + 1) * 512],
                                        mat[:, c, :],
                                        v16[c][:, half * 512 : (half + 1) * 512],
                                        start=(c == 0),
                                        stop=(c == 15),
                                    )
                            ao = wkpool.tile([128, H], F32, name="ao")
                            nc.vector.tensor_copy(ao[:], pa[:])
                            nc.sync.dma_start(
                                od.ap()[qrow : qrow + 128, :], ao[:]
                            )
    _split_excess_waits(nc)
    return nc


_NC_CACHE = []


def kernel(input_Q, input_K, input_V, w_omega, mask):
    if not _NC_CACHE:
        _NC_CACHE.append(build_nc())
    nc = _NC_CACHE[0]
    in_maps = []
    for b in range(NB):
        in_maps.append(
            {
                "q": np.ascontiguousarray(input_Q[b], np.float32),
                "k": np.ascontiguousarray(input_K[b], np.float32),
                "v": np.ascontiguousarray(input_V[b], np.float32),
                "w": np.ascontiguousarray(w_omega, np.float32),
                "mask": np.ascontiguousarray(
                    mask[b].reshape(1, LK), np.int32
                ),
            }
        )
    res = run_bass_kernel_spmd(nc, in_maps, core_ids=list(range(NB)))
    att_sm = np.stack([res.results[b]["o_sm"] for b in range(NB)])
    att_sg = np.stack([res.results[b]["o_sg"] for b in range(NB)])
    p_sm = np.stack([res.results[b]["o_psm"] for b in range(NB)])
    p_sg = np.stack([res.results[b]["o_psg"] for b in range(NB)])
    return att_sm, att_sg, p_sm, p_sg
